# revision 1
# baseline (speedup 1.0000x reference)
"""Cumulative (running) group norm over the frame axis on 8 trn2 NeuronCores.

Input  x: [B=8, T=8192, C=512] f32, weight: [C] f32.
Sharding: data-parallel over B -> one example per core, SPMD (identical
program, per-core input slice).

Per-core algorithm (T=8192 frames, C=512 channels), all math in f32:
  - layout frames as [P=128, NT=64] (frame t = col*128 + p), x resident in
    SBUF as [128, 64, 512]
  - per-frame channel mean/var via bn_stats/bn_aggr -> mu[p,i], v[p,i]
  - running mean  m[t]   = cumsum(mu)[t] / (t+1)
  - q[t] = v[t] + (mu[t] - m[t])^2 ;  running var[t] = cumsum(q)[t] / (t+1)
    (algebraically equal to the reference's cumsum of squared deviations
     from the running mean, divided by C*(t+1))
  - out = (x - m) * 1/sqrt(var + eps) * weight
  cumsum over t decomposes into: intra-column cumsum along partitions
  (triangular-ones matmul), column totals broadcast to all partitions
  (all-ones matmul), and a cross-column running carry (tensor_tensor_scan
  along the free dim, carried between chunks via its `initial` operand).
"""

from contextlib import ExitStack

import numpy as np

import concourse.bacc as bacc
import concourse.bass as bass
import concourse.tile as tile
from concourse import mybir
from concourse.bass_utils import run_bass_kernel_spmd

B, T, C = 8, 8192, 512
P = 128            # SBUF partitions
NT = T // P        # 64 frame-columns per core
GRP = 4            # frame-columns per DMA (4 * 256KiB = 1MiB)
N_CHUNKS = 8       # column chunks (running-carry granularity)
EPS = 1e-5
F32 = mybir.dt.float32
ADD = mybir.AluOpType.add
MULT = mybir.AluOpType.mult


def _emit_consts(nc, tc, ctx, triu_in, ones_in, invc_in, w_in):
    singles = ctx.enter_context(tc.tile_pool(name="singles", bufs=1))
    triu = singles.tile([P, P], F32)
    ones = singles.tile([P, P], F32)
    invc = singles.tile([P, NT], F32)
    nc.sync.dma_start(out=triu, in_=triu_in[:, :])
    nc.sync.dma_start(out=ones, in_=ones_in[:, :])
    nc.sync.dma_start(out=invc, in_=invc_in[:, :])
    wb = None
    if w_in is not None:
        wb = singles.tile([P, C], F32)
        nc.sync.dma_start(out=wb, in_=w_in[:].to_broadcast((P, C)))
    eps_t = singles.tile([P, 1], F32)
    nc.vector.memset(eps_t, EPS)
    return triu, ones, invc, wb, eps_t


def _emit_body(nc, tc, ctx, x_in, out_ext, consts, grp, n_chunks, uid=""):
    """Emit one full normalization pass x_in -> out_ext (DRAM APs)."""
    GRP, N_CHUNKS = grp, n_chunks
    triu, ones, invc, wb, eps_t = consts
    CW = NT // N_CHUNKS

    # Round-robin big DMAs across the three issue paths (SP/ACT HWDGE +
    # Pool SWDGE): each queue runs one DMA instruction at a time, so a
    # single queue serializes per-DMA issue overhead with its own transfer.
    dma_engs = [nc.sync, nc.scalar, nc.gpsimd]
    dma_i = [0]

    def next_dma_eng():
        e = dma_engs[dma_i[0] % len(dma_engs)]
        dma_i[0] += 1
        return e

    big = ctx.enter_context(tc.tile_pool(name=f"big{uid}", bufs=1))
    stats = ctx.enter_context(tc.tile_pool(name=f"stats{uid}", bufs=8))
    mvs = ctx.enter_context(tc.tile_pool(name=f"mvs{uid}", bufs=2))
    sm = ctx.enter_context(tc.tile_pool(name=f"sm{uid}", bufs=3))
    psum = ctx.enter_context(tc.tile_pool(name=f"psum{uid}", bufs=2, space="PSUM"))

    xb = big.tile([P, NT, C], F32)

    zero2 = sm.tile([P, 2], F32)
    nc.vector.memset(zero2, 0.0)
    carry_mu = zero2[:, 0:1]
    carry_q = zero2[:, 1:2]

    for ch in range(N_CHUNKS):
        c0 = ch * CW
        # ---- stage A: load chunk + per-frame stats --------------------
        for g in range(c0 // GRP, (c0 + CW) // GRP):
            rows = x_in[g * GRP * P:(g + 1) * GRP * P, :]
            next_dma_eng().dma_start(
                out=xb[:, g * GRP:(g + 1) * GRP, :],
                in_=rows.rearrange("(j p) c -> p j c", j=GRP),
            )
        mv = mvs.tile([P, CW, 2], F32)
        for i in range(CW):
            st = stats.tile([P, 6], F32)
            nc.vector.bn_stats(out=st, in_=xb[:, c0 + i, :])
            nc.vector.bn_aggr(out=mv[:, i, :], in_=st)

        # ---- stage B: running stats over time -------------------------
        mu = sm.tile([P, CW], F32)
        vv = sm.tile([P, CW], F32)
        nc.vector.tensor_copy(out=mu, in_=mv[:, :, 0])
        nc.vector.tensor_copy(out=vv, in_=mv[:, :, 1])

        cs_mu = psum.tile([P, CW], F32)
        col_mu = psum.tile([P, CW], F32)
        nc.tensor.matmul(cs_mu, triu, mu, start=True, stop=True)
        nc.tensor.matmul(col_mu, ones, mu, start=True, stop=True)
        # E[:, i] = carry + sum_{i' <= i} col_mu[:, i']   (inclusive)
        E = sm.tile([P, CW], F32)
        nc.vector.tensor_tensor_scan(
            E, ones[:, :CW], col_mu, carry_mu, MULT, ADD)
        stot = sm.tile([P, CW], F32)
        nc.vector.tensor_scalar_add(stot[:, 0:1], cs_mu[:, 0:1], carry_mu)
        if CW > 1:
            nc.vector.tensor_tensor(
                out=stot[:, 1:], in0=cs_mu[:, 1:], in1=E[:, :CW - 1], op=ADD)
        carry_mu = E[:, CW - 1:CW]
        m = sm.tile([P, CW], F32)
        nc.vector.tensor_mul(out=m, in0=stot, in1=invc[:, c0:c0 + CW])

        d = sm.tile([P, CW], F32)
        q = sm.tile([P, CW], F32)
        nc.vector.tensor_sub(out=d, in0=mu, in1=m)
        nc.vector.tensor_mul(out=q, in0=d, in1=d)
        nc.vector.tensor_add(out=q, in0=q, in1=vv)

        cs_q = psum.tile([P, CW], F32)
        col_q = psum.tile([P, CW], F32)
        nc.tensor.matmul(cs_q, triu, q, start=True, stop=True)
        nc.tensor.matmul(col_q, ones, q, start=True, stop=True)
        Eq = sm.tile([P, CW], F32)
        nc.vector.tensor_tensor_scan(
            Eq, ones[:, :CW], col_q, carry_q, MULT, ADD)
        vtot = sm.tile([P, CW], F32)
        nc.vector.tensor_scalar_add(vtot[:, 0:1], cs_q[:, 0:1], carry_q)
        if CW > 1:
            nc.vector.tensor_tensor(
                out=vtot[:, 1:], in0=cs_q[:, 1:], in1=Eq[:, :CW - 1], op=ADD)
        carry_q = Eq[:, CW - 1:CW]
        var = sm.tile([P, CW], F32)
        nc.vector.tensor_mul(out=var, in0=vtot, in1=invc[:, c0:c0 + CW])

        rstd = sm.tile([P, CW], F32)
        nc.scalar.activation(
            out=rstd, in_=var, func=mybir.ActivationFunctionType.Sqrt,
            bias=eps_t[:, 0:1])
        nc.vector.reciprocal(out=rstd, in_=rstd)
        nmr = sm.tile([P, CW], F32)
        nc.vector.tensor_mul(out=nmr, in0=m, in1=rstd)
        nc.scalar.mul(out=nmr, in_=nmr, mul=-1.0)

        # ---- stage C: normalize + store -------------------------------
        for i in range(CW):
            nc.scalar.activation(
                out=xb[:, c0 + i, :], in_=xb[:, c0 + i, :],
                func=mybir.ActivationFunctionType.Identity,
                bias=nmr[:, i:i + 1], scale=rstd[:, i:i + 1])
            if wb is not None:
                nc.vector.tensor_mul(
                    out=xb[:, c0 + i, :], in0=xb[:, c0 + i, :], in1=wb)
        for g in range(c0 // GRP, (c0 + CW) // GRP):
            rows = out_ext[g * GRP * P:(g + 1) * GRP * P, :]
            next_dma_eng().dma_start(
                out=rows.rearrange("(j p) c -> p j c", j=GRP),
                in_=xb[:, g * GRP:(g + 1) * GRP, :],
            )


def _build(apply_weight: bool, grp: int = None, n_chunks: int = None) -> bass.Bass:
    grp = grp if grp is not None else GRP
    n_chunks = n_chunks if n_chunks is not None else N_CHUNKS
    nc = bacc.Bacc(None, target_bir_lowering=False, debug=False)
    x_in = nc.declare_dram_parameter("x", [T, C], F32, isOutput=False)
    triu_in = nc.declare_dram_parameter("triu", [P, P], F32, isOutput=False)
    ones_in = nc.declare_dram_parameter("ones", [P, P], F32, isOutput=False)
    invc_in = nc.declare_dram_parameter("invcnt", [P, NT], F32, isOutput=False)
    w_in = None
    if apply_weight:
        w_in = nc.declare_dram_parameter("weight", [C], F32, isOutput=False)
    out_ext = nc.declare_dram_parameter("out", [T, C], F32, isOutput=True)

    with tile.TileContext(nc) as tc, ExitStack() as ctx:
        consts = _emit_consts(nc, tc, ctx, triu_in, ones_in, invc_in, w_in)
        _emit_body(nc, tc, ctx, x_in, out_ext, consts, grp, n_chunks)
    nc.compile()
    return nc


def _build_chained(k_iters: int, apply_weight: bool = False,
                   grp: int = None, n_chunks: int = None) -> bass.Bass:
    """k_iters sequential executions chained through internal DRAM tiles
    (for marginal-time measurement)."""
    grp = grp if grp is not None else GRP
    n_chunks = n_chunks if n_chunks is not None else N_CHUNKS
    nc = bacc.Bacc(None, target_bir_lowering=False, debug=False)
    x_in = nc.declare_dram_parameter("x", [T, C], F32, isOutput=False)
    triu_in = nc.declare_dram_parameter("triu", [P, P], F32, isOutput=False)
    ones_in = nc.declare_dram_parameter("ones", [P, P], F32, isOutput=False)
    invc_in = nc.declare_dram_parameter("invcnt", [P, NT], F32, isOutput=False)
    w_in = None
    if apply_weight:
        w_in = nc.declare_dram_parameter("weight", [C], F32, isOutput=False)
    out_ext = nc.declare_dram_parameter("out", [T, C], F32, isOutput=True)

    with tile.TileContext(nc) as tc, ExitStack() as octx:
        consts = _emit_consts(nc, tc, octx, triu_in, ones_in, invc_in, w_in)
        dpool = octx.enter_context(tc.tile_pool(name="dchain", bufs=2,
                                                space="DRAM"))
        src = x_in
        for k in range(k_iters):
            dst = out_ext if k == k_iters - 1 else dpool.tile([T, C], F32)
            with ExitStack() as ictx:
                _emit_body(nc, tc, ictx, src, dst, consts, grp, n_chunks,
                           uid=f"_k{k}")
            src = dst
    nc.compile()
    return nc


def _build_loop(k_iters: int, apply_weight: bool = False,
                grp: int = None, n_chunks: int = None) -> bass.Bass:
    """One body inside a dynamic For_i loop of k_iters, normalizing an
    internal DRAM buffer in place. NEFF size is independent of k_iters, so
    T(k2) - T(k1) isolates pure per-iteration execution time."""
    grp = grp if grp is not None else GRP
    n_chunks = n_chunks if n_chunks is not None else N_CHUNKS
    nc = bacc.Bacc(None, target_bir_lowering=False, debug=False)
    x_in = nc.declare_dram_parameter("x", [T, C], F32, isOutput=False)
    triu_in = nc.declare_dram_parameter("triu", [P, P], F32, isOutput=False)
    ones_in = nc.declare_dram_parameter("ones", [P, P], F32, isOutput=False)
    invc_in = nc.declare_dram_parameter("invcnt", [P, NT], F32, isOutput=False)
    w_in = None
    if apply_weight:
        w_in = nc.declare_dram_parameter("weight", [C], F32, isOutput=False)
    out_ext = nc.declare_dram_parameter("out", [T, C], F32, isOutput=True)
    d = nc.dram_tensor("dwork", [T, C], F32)

    with tile.TileContext(nc) as tc, ExitStack() as octx:
        consts = _emit_consts(nc, tc, octx, triu_in, ones_in, invc_in, w_in)
        nc.sync.dma_start(out=d[:, :], in_=x_in[:, :])
        with tc.For_i(0, k_iters, 1):
            with ExitStack() as ictx:
                _emit_body(nc, tc, ictx, d, d, consts, grp, n_chunks,
                           uid="_L")
        nc.sync.dma_start(out=out_ext[:, :], in_=d[:, :])
    nc.compile()
    return nc


def _build_loop_timing(k_iters: int, grp: int = None,
                       n_chunks: int = None) -> bass.Bass:
    """Timing-only: like _build_loop but with tiny I/O so per-call wall
    time is dispatch + execution, not 256MiB tunnel transfers. The loop
    normalizes an uninitialized internal DRAM buffer (zeros -> stays
    finite)."""
    grp = grp if grp is not None else GRP
    n_chunks = n_chunks if n_chunks is not None else N_CHUNKS
    nc = bacc.Bacc(None, target_bir_lowering=False, debug=False)
    triu_in = nc.declare_dram_parameter("triu", [P, P], F32, isOutput=False)
    ones_in = nc.declare_dram_parameter("ones", [P, P], F32, isOutput=False)
    invc_in = nc.declare_dram_parameter("invcnt", [P, NT], F32, isOutput=False)
    out_ext = nc.declare_dram_parameter("out", [P, 4], F32, isOutput=True)
    d = nc.dram_tensor("dwork", [T, C], F32)

    with tile.TileContext(nc) as tc, ExitStack() as octx:
        consts = _emit_consts(nc, tc, octx, triu_in, ones_in, invc_in, None)
        with tc.For_i(0, k_iters, 1):
            with ExitStack() as ictx:
                _emit_body(nc, tc, ictx, d, d, consts, grp, n_chunks,
                           uid="_L")
        nc.sync.dma_start(out=out_ext[:, :], in_=d[0:P, 0:4])
    nc.compile()
    return nc


_PROGRAMS: dict[bool, bass.Bass] = {}


def _consts() -> dict[str, np.ndarray]:
    triu = np.triu(np.ones((P, P), dtype=np.float32))
    ones = np.ones((P, P), dtype=np.float32)
    t = (np.arange(NT, dtype=np.float32)[None, :] * P
         + np.arange(P, dtype=np.float32)[:, None])
    invcnt = (1.0 / (t + 1.0)).astype(np.float32)
    return {"triu": triu, "ones": ones, "invcnt": invcnt}


def _run(inputs: dict, **run_kwargs):
    x = np.ascontiguousarray(np.asarray(inputs["x"], dtype=np.float32))
    w = np.ascontiguousarray(np.asarray(inputs["weight"], dtype=np.float32))
    apply_weight = not bool(np.all(w == 1.0))
    if apply_weight not in _PROGRAMS:
        _PROGRAMS[apply_weight] = _build(apply_weight)
    nc = _PROGRAMS[apply_weight]
    consts = _consts()
    in_maps = []
    for b in range(B):
        m = {"x": x[b], **consts}
        if apply_weight:
            m["weight"] = w
        in_maps.append(m)
    res = run_bass_kernel_spmd(nc, in_maps, core_ids=list(range(B)),
                               **run_kwargs)
    out = np.stack([res.results[b]["out"] for b in range(B)], axis=0)
    return out, res


def kernel(**inputs) -> np.ndarray:
    in_dtype = np.asarray(inputs["x"]).dtype
    out, _ = _run(inputs)
    return out.astype(in_dtype)



# revision 2
# speedup vs baseline: 1.1250x; 1.1250x over previous
"""Cumulative (running) group norm over the frame axis on 8 trn2 NeuronCores.

Input  x: [B=8, T=8192, C=512] f32, weight: [C] f32.
Sharding: data-parallel over B -> one example per core, SPMD (identical
program, per-core input slice).

Per-core algorithm (T=8192 frames, C=512 channels), all math in f32:
  - layout frames as [P=128, NT=64] (frame t = col*128 + p), x resident in
    SBUF as [128, 64, 512]
  - per-frame channel mean/var via bn_stats/bn_aggr -> mu[t], v[t]
  - running mean m[t] = cumsum(mu)[t] / (t+1)
  - running var[t]    = cumsum(mu^2 + v)[t] / (t+1) - m[t]^2
    (cumsum(mu^2+v)/(t+1) = running E[x^2]; algebraically equal to the
     reference's cumsum of squared deviations from the running mean)
  - out = (x - m) * 1/sqrt(var + eps) * weight
  The two cumsums run fused side by side in one [P, 2*CW] tile: an
  intra-column prefix (triangular-ones matmul over partitions), column
  totals (all-ones matmul), and a cross-column running carry
  (tensor_tensor_scan along the free dim, chained between chunks).

DMA strategy (the kernel is HBM-bound: 16 MiB in + 16 MiB out per core at
~358 GB/s is ~92 us):
  - per-column 256 KiB transfers (2 KiB per partition, line-rate
    descriptors); small units keep per-DMA completion latency low so
    compute starts ~10 us earlier than with 1 MiB groups
  - all loads issue on the SP HWDGE ring (nc.sync): FIFO within the ring
    means completion order == issue order and nothing else queues ahead
  - all stores issue on the SWDGE ring (nc.gpsimd): a store waiting for
    its column's normalize can never head-of-line block a load, and the
    ACT sequencer (which runs the normalize) never issues DMA descriptors
  - chunk sizes taper at both ends: early stores start sooner, and the
    tail after the last load (last chunk's stats -> cumsum -> normalize ->
    store) stays short.
"""

from contextlib import ExitStack

import numpy as np

import concourse.bacc as bacc
import concourse.bass as bass
import concourse.tile as tile
from concourse import mybir
from concourse.bass_utils import run_bass_kernel_spmd

B, T, C = 8, 8192, 512
P = 128            # SBUF partitions
NT = T // P        # 64 frame-columns per core
CHUNKS = (4, 4, 8, 8, 8, 8, 8, 8, 4, 2, 2)   # sum == NT
EPS = 1e-5
F32 = mybir.dt.float32
ADD = mybir.AluOpType.add
SUB = mybir.AluOpType.subtract
MULT = mybir.AluOpType.mult


def _emit_consts(nc, tc, ctx, triu_in, ones_in, invc_in, w_in):
    singles = ctx.enter_context(tc.tile_pool(name="singles", bufs=1))
    triu = singles.tile([P, P], F32)
    ones = singles.tile([P, P], F32)
    invc = singles.tile([P, NT], F32)
    # Consts ride the SWDGE ring ahead of any store; they land well before
    # the first chunk's cumsum needs them and keep the SP ring free for the
    # first x loads.
    nc.gpsimd.dma_start(out=triu, in_=triu_in[:, :])
    nc.gpsimd.dma_start(out=ones, in_=ones_in[:, :])
    nc.gpsimd.dma_start(out=invc, in_=invc_in[:, :])
    wb = None
    if w_in is not None:
        wb = singles.tile([P, C], F32)
        nc.gpsimd.dma_start(out=wb, in_=w_in[:].to_broadcast((P, C)))
    eps_t = singles.tile([P, 1], F32)
    nc.vector.memset(eps_t, EPS)
    zero2 = singles.tile([P, 2], F32)
    nc.vector.memset(zero2, 0.0)
    return triu, ones, invc, wb, eps_t, zero2


def _emit_body(nc, tc, ctx, x_in, out_ext, consts):
    triu, ones, invc, wb, eps_t, zero2 = consts

    big = ctx.enter_context(tc.tile_pool(name="big", bufs=1))
    stats = ctx.enter_context(tc.tile_pool(name="stats", bufs=8))
    mvs = ctx.enter_context(tc.tile_pool(name="mvs", bufs=2))
    sm = ctx.enter_context(tc.tile_pool(name="sm", bufs=4))
    psum = ctx.enter_context(tc.tile_pool(name="psum", bufs=2, space="PSUM"))

    xb = big.tile([P, NT, C], F32)

    carry_mu = zero2[:, 0:1]
    carry_s2 = zero2[:, 1:2]

    c0 = 0
    for cw in CHUNKS:
        # ---- loads: one 256 KiB DMA per frame-column (SP ring) ---------
        for i in range(cw):
            col = c0 + i
            nc.sync.dma_start(
                out=xb[:, col, :], in_=x_in[col * P:(col + 1) * P, :])

        # ---- per-frame stats (DVE) ------------------------------------
        mv = mvs.tile([P, cw, 2], F32)
        for i in range(cw):
            st = stats.tile([P, 6], F32)
            nc.vector.bn_stats(out=st, in_=xb[:, c0 + i, :])
            nc.vector.bn_aggr(out=mv[:, i, :], in_=st)

        # ---- fused running stats: [mu | mu^2 + v] cumsum over time ----
        both = sm.tile([P, 2 * cw], F32)
        nc.vector.tensor_copy(out=both[:, :cw], in_=mv[:, :, 0])
        nc.vector.tensor_mul(out=both[:, cw:], in0=mv[:, :, 0], in1=mv[:, :, 0])
        nc.vector.tensor_add(out=both[:, cw:], in0=both[:, cw:], in1=mv[:, :, 1])

        cs = psum.tile([P, 2 * cw], F32)
        colt = psum.tile([P, 2 * cw], F32)
        nc.tensor.matmul(cs, triu, both, start=True, stop=True)
        nc.tensor.matmul(colt, ones, both, start=True, stop=True)

        # E[:, i] = carry + sum_{i' <= i} colt[:, i']   (inclusive)
        E = sm.tile([P, 2 * cw], F32)
        nc.vector.tensor_tensor_scan(
            E[:, :cw], ones[:, :cw], colt[:, :cw], carry_mu, MULT, ADD)
        nc.vector.tensor_tensor_scan(
            E[:, cw:], ones[:, :cw], colt[:, cw:], carry_s2, MULT, ADD)
        tot = sm.tile([P, 2 * cw], F32)
        nc.vector.tensor_scalar_add(tot[:, 0:1], cs[:, 0:1], carry_mu)
        nc.vector.tensor_scalar_add(tot[:, cw:cw + 1], cs[:, cw:cw + 1], carry_s2)
        if cw > 1:
            nc.vector.tensor_tensor(
                out=tot[:, 1:cw], in0=cs[:, 1:cw], in1=E[:, :cw - 1], op=ADD)
            nc.vector.tensor_tensor(
                out=tot[:, cw + 1:], in0=cs[:, cw + 1:], in1=E[:, cw:2 * cw - 1],
                op=ADD)
        carry_mu = E[:, cw - 1:cw]
        carry_s2 = E[:, 2 * cw - 1:2 * cw]

        m = sm.tile([P, cw], F32)
        nc.vector.tensor_mul(out=m, in0=tot[:, :cw], in1=invc[:, c0:c0 + cw])
        var = sm.tile([P, cw], F32)
        nc.vector.tensor_mul(out=var, in0=tot[:, cw:], in1=invc[:, c0:c0 + cw])
        msq = sm.tile([P, cw], F32)
        nc.vector.tensor_mul(out=msq, in0=m, in1=m)
        nc.vector.tensor_sub(out=var, in0=var, in1=msq)

        rstd = sm.tile([P, cw], F32)
        nc.scalar.activation(
            out=rstd, in_=var, func=mybir.ActivationFunctionType.Sqrt,
            bias=eps_t[:, 0:1])
        nc.vector.reciprocal(out=rstd, in_=rstd)
        nmr = sm.tile([P, cw], F32)
        nc.vector.scalar_tensor_tensor(
            out=nmr, in0=m, scalar=-1.0, in1=rstd, op0=MULT, op1=MULT)

        # ---- normalize (ACT) + store (SWDGE ring), per column ---------
        for i in range(cw):
            col = c0 + i
            nc.scalar.activation(
                out=xb[:, col, :], in_=xb[:, col, :],
                func=mybir.ActivationFunctionType.Identity,
                bias=nmr[:, i:i + 1], scale=rstd[:, i:i + 1])
            if wb is not None:
                nc.vector.tensor_mul(
                    out=xb[:, col, :], in0=xb[:, col, :], in1=wb)
            nc.gpsimd.dma_start(
                out=out_ext[col * P:(col + 1) * P, :], in_=xb[:, col, :])
        c0 += cw


def _build(apply_weight: bool) -> bass.Bass:
    assert sum(CHUNKS) == NT
    nc = bacc.Bacc(None, target_bir_lowering=False, debug=False)
    x_in = nc.declare_dram_parameter("x", [T, C], F32, isOutput=False)
    triu_in = nc.declare_dram_parameter("triu", [P, P], F32, isOutput=False)
    ones_in = nc.declare_dram_parameter("ones", [P, P], F32, isOutput=False)
    invc_in = nc.declare_dram_parameter("invcnt", [P, NT], F32, isOutput=False)
    w_in = None
    if apply_weight:
        w_in = nc.declare_dram_parameter("weight", [C], F32, isOutput=False)
    out_ext = nc.declare_dram_parameter("out", [T, C], F32, isOutput=True)

    with tile.TileContext(nc) as tc, ExitStack() as ctx:
        consts = _emit_consts(nc, tc, ctx, triu_in, ones_in, invc_in, w_in)
        _emit_body(nc, tc, ctx, x_in, out_ext, consts)
    nc.compile()
    return nc


_PROGRAMS: dict[bool, bass.Bass] = {}


def _consts() -> dict[str, np.ndarray]:
    triu = np.triu(np.ones((P, P), dtype=np.float32))
    ones = np.ones((P, P), dtype=np.float32)
    t = (np.arange(NT, dtype=np.float32)[None, :] * P
         + np.arange(P, dtype=np.float32)[:, None])
    invcnt = (1.0 / (t + 1.0)).astype(np.float32)
    return {"triu": triu, "ones": ones, "invcnt": invcnt}


def _run(inputs: dict, **run_kwargs):
    x = np.ascontiguousarray(np.asarray(inputs["x"], dtype=np.float32))
    w = np.ascontiguousarray(np.asarray(inputs["weight"], dtype=np.float32))
    apply_weight = not bool(np.all(w == 1.0))
    if apply_weight not in _PROGRAMS:
        _PROGRAMS[apply_weight] = _build(apply_weight)
    nc = _PROGRAMS[apply_weight]
    consts = _consts()
    in_maps = []
    for b in range(B):
        m = {"x": x[b], **consts}
        if apply_weight:
            m["weight"] = w
        in_maps.append(m)
    res = run_bass_kernel_spmd(nc, in_maps, core_ids=list(range(B)),
                               **run_kwargs)
    out = np.stack([res.results[b]["out"] for b in range(B)], axis=0)
    return out, res


def kernel(**inputs) -> np.ndarray:
    in_dtype = np.asarray(inputs["x"]).dtype
    out, _ = _run(inputs)
    return out.astype(in_dtype)
